# revision 1
# baseline (speedup 1.0000x reference)
"""Trainium2 Bass kernel: Conformer block (B=8, N=512, DIM=512, H=8, DH=64, FF=2048, CIN=1024, K=31).

Sharding: pure data-parallel over batch — each of the 8 NeuronCores processes one
batch item with the full weight set (no collectives).

Layout: activations are kept FEATURE-major ([feature, time] = x.T) on chip so that
chained matmuls need no transposes.  LayerNorm reductions over features become
ones-vector matmuls on the PE.

Performance structure (~400us baseline -> ~310us):
  - All small constants live in one f32 blob (1 DMA); x/out are single 3D-AP
    DMAs; every weight matrix loads with 1-4 large dma_starts spread over the
    SP/Act/Pool DMA queues (vs ~180 tiny transfers), prefetched a phase ahead.
  - All large matmuls run with bf16 stationaries AND bf16 moving data (weights
    converted on host, LN output z written bf16): bf16 LDWEIGHTS is ~3x faster
    than f32r and HBM weight traffic halves.  Residual stream stays f32.
  - The relative-position shift-gather bounce (qr = q @ relT -> per-head DRAM
    scratch) is bf16; the readback uses the DMA XBAR (transpose=True) to
    deliver rel ALREADY TRANSPOSED, so the per-head 16 PE transposes collapse
    into 4 identity-stationary accumulate matmuls.  One write + one
    transpose-gather dma_start per head.
  - Attention is software-pipelined in three stages (qr bounce / kq+rel+exp /
    attn@v+normalize) with lookahead 3 so the PE never waits on the bounce or
    the Act-engine exps.
  - q/k tiles are zero-padded to K=128 per head parity: the hardware activity
    manager (HAM, 3.4us quanta) duty-cycles the PE to 4/8 when the reported
    MAC utilization is low; K=64 matmuls looked half-idle and pinned the whole
    attention phase at half clock.
  - LayerNorm stats use a 1/DIM-scaled ones stationary so mean/E[x^2] come out
    of PSUM directly, shortening the serial cross-engine chain at each of the
    4 LN phase boundaries.
  - The depthwise conv runs as 31 PSUM-accumulated diagonal matmuls per
    128-channel block in bf16 (streams at the full 2.4 GHz PE rate).
"""

import sys

for _p in ("/opt/trn_rl_repo", "/root/.axon_site/_ro/trn_rl_repo"):
    if _p not in sys.path:
        sys.path.insert(0, _p)

import numpy as np

B, N, DIM, H, DH, MULT, EXP, KW, MAXP = 8, 512, 512, 8, 64, 4, 2, 31, 512
INNER = H * DH
FF = DIM * MULT
CIN = DIM * EXP
EPS = 1e-5
P = 128
DT = DIM // P      # 4  feature tiles of the residual stream
FT = FF // P       # 16 ff hidden tiles
CT = CIN // P      # 8  conv channel tiles
NCORES = 8
PAD = KW - 1       # 30 causal pad
QRW = 2 * MAXP + 1  # 1025 scratch row width

# constants blob column layout: name -> (offset, width)
_CST_SECTS = [
    ("b1", FT), ("b3", FT), ("b2", DT), ("b4", DT), ("bq", DT), ("bk", DT),
    ("bo", DT), ("c2b", DT), ("c1a", CT), ("c1g", CT), ("bns", CT), ("bnt", CT),
    ("png", DT), ("pnb", DT), ("bvb", INNER),
]
CST_OFF = {}
_o = 0
for _n, _w in _CST_SECTS:
    CST_OFF[_n] = (_o, _w)
    _o += _w
CSTW = _o  # 736


def build(split_waits=True):
    """Build the single-core Bass module (SPMD: same NEFF on all 8 cores)."""
    import concourse.bass as bass
    import concourse.mybir as mybir
    import concourse.tile as tile

    F32 = mybir.dt.float32
    F32R = mybir.dt.float32r
    BF16 = mybir.dt.bfloat16
    AF = mybir.ActivationFunctionType
    AL = mybir.AluOpType

    nc = bass.Bass()

    # ---------------- I/O ----------------
    xT_d = nc.dram_tensor("xT", [DIM, N], F32R, kind="ExternalInput")
    cst_d = nc.dram_tensor("cst", [P, CSTW], F32, kind="ExternalInput")
    cstr_d = nc.dram_tensor("cstr", [P, 3 * P], F32R, kind="ExternalInput")
    idbf_d = nc.dram_tensor("idbf", [P, P], BF16, kind="ExternalInput")
    relT_d = nc.dram_tensor("relT", [P, QRW], BF16, kind="ExternalInput")
    w1_d = nc.dram_tensor("w1", [DIM, FF], BF16, kind="ExternalInput")
    w2_d = nc.dram_tensor("w2", [FF, DIM], BF16, kind="ExternalInput")
    wq_d = nc.dram_tensor("wq", [DIM, INNER], BF16, kind="ExternalInput")
    wk_d = nc.dram_tensor("wk", [DIM, INNER], BF16, kind="ExternalInput")
    wv_d = nc.dram_tensor("wv", [DIM, INNER], BF16, kind="ExternalInput")
    wo_d = nc.dram_tensor("wo", [INNER, DIM], BF16, kind="ExternalInput")
    c1_d = nc.dram_tensor("c1", [DIM, 2 * CIN], BF16, kind="ExternalInput")
    dwd_d = nc.dram_tensor("dwdiag", [CT, P, KW * P], BF16, kind="ExternalInput")
    c2_d = nc.dram_tensor("c2", [CIN, DIM], BF16, kind="ExternalInput")
    w3_d = nc.dram_tensor("w3", [DIM, FF], BF16, kind="ExternalInput")
    w4_d = nc.dram_tensor("w4", [FF, DIM], BF16, kind="ExternalInput")
    outT_d = nc.dram_tensor("outT", [DIM, N], F32, kind="ExternalOutput")

    qr_ds = [nc.dram_tensor(f"qr_scr{h}", [N, QRW], BF16, kind="Internal")
             for h in range(H)]

    def r32(ap):
        return ap.bitcast(F32R)

    with tile.TileContext(nc) as tc:
        with (
            nc.allow_low_precision(reason="fp32r/bf16 matmul feeds"),
            tc.tile_pool(name="cst", bufs=1) as cst,
            tc.tile_pool(name="sb", bufs=2) as sb,
            tc.tile_pool(name="ps", bufs=2, space="PSUM") as psp,
        ):

            # ---------------- x first (LN1 critical path) ----------------
            xt4 = sb.tile([P, DT, N], F32R, tag="x4", bufs=1)
            nc.sync.dma_start(
                xt4[:, :, :],
                bass.AP(xT_d, 0, [[N, P], [P * N, DT], [1, N]]))
            xs = [xt4[:, kt, :] for kt in range(DT)]

            # ---------------- constants ----------------
            cstt = cst.tile([P, CSTW], F32, tag="cstt")
            nc.scalar.dma_start(cstt[:, :], cst_d[:, :])
            cstr = cst.tile([P, 3 * P], F32R, tag="cstr")
            nc.sync.dma_start(cstr[:, :], cstr_d[:, :])
            idbf = cst.tile([P, P], BF16, tag="idbf")
            nc.scalar.dma_start(idbf[:, :], idbf_d[:, :])

            def cv(name):
                off, w = CST_OFF[name]
                return cstt[:, off:off + w]

            ones_v = cstr[:, 0:P]    # [P, 128] f32r ones
            ident_v = cstr[:, P:2 * P]  # [P, 128] f32r identity
            oosd_v = cstr[:, 2 * P:3 * P]  # [P, 128] f32r 1/DIM

            # ---------------- ff1 weights (sync + scalar queues) ----------
            w1ts = []
            for kt in range(DT):
                wt = sb.tile([P, FF], BF16, tag="wst", bufs=5, name=f"w1t{kt}")
                eng = nc.sync if kt < 2 else nc.scalar
                eng.dma_start(wt[:, :], w1_d[kt * P:(kt + 1) * P, :])
                w1ts.append(wt)
            w2ts = []
            for q in range(4):
                wt = sb.tile([P, 4, DIM], BF16, tag="w2t", bufs=4, name=f"w2t{q}")
                nc.scalar.dma_start(
                    wt[:, :, :],
                    bass.AP(w2_d, q * 4 * P * DIM,
                            [[DIM, P], [P * DIM, 4], [1, DIM]]))
                w2ts.append(wt)

            # ---------------- helpers ----------------
            def layer_norm_rc(xin):
                """LN stats over the partition (feature) axis.

                Returns r_b, c_b [128, 512] tiles with z = x*r_b + c_b."""
                # scaled-ones (1/DIM) stationary: psum holds mean / E[x^2]
                ps_sum = psp.tile([P, N], F32, tag="s1", bufs=1)
                for kt in range(DT):
                    nc.tensor.matmul(ps_sum[:, :], r32(oosd_v), xin[kt][:, :],
                                     start=(kt == 0), stop=(kt == DT - 1))
                ps_sq = psp.tile([P, N], F32, tag="s2", bufs=1)
                for kt in range(DT):
                    xsq = sb.tile([P, N], F32R, tag="tmp", bufs=3)
                    nc.scalar.square(xsq[:, :], xin[kt][:, :])
                    nc.tensor.matmul(ps_sq[:, :], r32(oosd_v), xsq[:, :],
                                     start=(kt == 0), stop=(kt == DT - 1))
                m2 = sb.tile([P, N], F32, tag="tmp", bufs=3)
                nc.scalar.square(m2[:, :], ps_sum[:, :])
                veps = sb.tile([P, N], F32, tag="tmp", bufs=3)
                nc.vector.scalar_tensor_tensor(veps[:, :], ps_sq[:, :], EPS,
                                               m2[:, :], AL.add, AL.subtract)
                lnv = sb.tile([P, N], F32, tag="tmp", bufs=3)
                nc.scalar.activation(lnv[:, :], veps[:, :], AF.Ln)
                r_b = sb.tile([P, N], F32, tag="r_b", bufs=1)
                nc.scalar.activation(r_b[:, :], lnv[:, :], AF.Exp, scale=-0.5)
                c_b = sb.tile([P, N], F32, tag="c_b", bufs=1)
                nc.vector.scalar_tensor_tensor(c_b[:, :], ps_sum[:, :], -1.0,
                                               r_b[:, :], AL.mult, AL.mult)
                return r_b, c_b

            def ln_apply(xin, r_b, c_b):
                zs = []
                for kt in range(DT):
                    t = sb.tile([P, N], F32, tag="lnt", bufs=2)
                    nc.vector.tensor_mul(t[:, :], xin[kt][:, :], r_b[:, :])
                    z = sb.tile([P, N], BF16, tag="z", bufs=4)
                    nc.vector.tensor_add(z[:, :], t[:, :], c_b[:, :])
                    zs.append(z)
                return zs

            def ff_block(xin, w1tiles, b1v, w2tiles, b2v):
                """x + 0.5*ff(LN(x)); returns new residual tiles."""
                r_b, c_b = layer_norm_rc(xin)
                zs = ln_apply(xin, r_b, c_b)
                h1s = []
                for mt in range(FT):
                    ph = psp.tile([P, N], F32, tag="acc", bufs=4)
                    for kt in range(DT):
                        nc.tensor.matmul(ph[:, :],
                                         w1tiles[kt][:, mt * P:(mt + 1) * P],
                                         zs[kt][:, :],
                                         start=(kt == 0), stop=(kt == DT - 1))
                    sig = sb.tile([P, N], F32, tag="tmp", bufs=3)
                    nc.scalar.activation(sig[:, :], ph[:, :], AF.Sigmoid,
                                         bias=b1v[:, mt:mt + 1], scale=1.0)
                    hs = sb.tile([P, N], BF16, tag="h1s", bufs=16)
                    nc.vector.scalar_tensor_tensor(hs[:, :], ph[:, :],
                                                   b1v[:, mt:mt + 1], sig[:, :],
                                                   AL.add, AL.mult)
                    h1s.append(hs)
                pys = [psp.tile([P, N], F32, tag="acc", bufs=4, name=f"pys{i}")
                       for i in range(DT)]
                for kt in range(FT):
                    wv_ = w2tiles[kt // 4][:, kt % 4, :]
                    for mt in range(DT):
                        nc.tensor.matmul(pys[mt][:, :], wv_[:, mt * P:(mt + 1) * P],
                                         h1s[kt][:, :],
                                         start=(kt == 0), stop=(kt == FT - 1))
                xo = []
                for mt in range(DT):
                    t = sb.tile([P, N], F32R, tag="x", bufs=6)
                    nc.vector.scalar_tensor_tensor(t[:, :], pys[mt][:, :],
                                                   b2v[:, mt:mt + 1], xin[mt][:, :],
                                                   AL.add, AL.add)
                    xo.append(t)
                return xo

            # ================= ff1 =================
            x1 = ff_block(xs, w1ts, cv("b1"), w2ts, cv("b2"))

            # prefetch attention weights on the sync queue (runs during ff1)
            wq4 = sb.tile([P, DT, INNER], BF16, tag="wst", bufs=5)
            nc.sync.dma_start(wq4[:, :, :],
                              bass.AP(wq_d, 0, [[INNER, P], [P * INNER, DT],
                                                [1, INNER]]))
            wk4 = sb.tile([P, DT, INNER], BF16, tag="wst", bufs=5)
            nc.sync.dma_start(wk4[:, :, :],
                              bass.AP(wk_d, 0, [[INNER, P], [P * INNER, DT],
                                                [1, INNER]]))
            wv4 = sb.tile([P, DT, INNER], BF16, tag="wst", bufs=5)
            nc.sync.dma_start(wv4[:, :, :],
                              bass.AP(wv_d, 0, [[INNER, P], [P * INNER, DT],
                                                [1, INNER]]))
            relTt = sb.tile([P, QRW], BF16, tag="wst", bufs=5)
            nc.sync.dma_start(relTt[:, :], relT_d[:, :])
            wo4 = sb.tile([P, DT, INNER], BF16, tag="w2t", bufs=4)
            nc.scalar.dma_start(wo4[:, :, :],
                                bass.AP(wo_d, 0, [[INNER, P], [P * INNER, DT],
                                                  [1, INNER]]))

            # ================= attention =================
            r_b, c_b = layer_norm_rc(x1)
            zs = ln_apply(x1, r_b, c_b)

            # q/k tiles per (pair, parity) with the other head's rows ZERO so
            # every attention matmul runs at full K=128 MAC rate (HAM keeps
            # the PE at full duty only when reported utilization is high).
            def proj_qk_padded(wt4, bias_v, tag):
                outs = []
                for mt in range(DT):
                    pq = psp.tile([P, N], F32, tag="mm", bufs=2)
                    for kt in range(DT):
                        nc.tensor.matmul(pq[:, :],
                                         wt4[:, kt, mt * P:(mt + 1) * P],
                                         zs[kt][:, :],
                                         start=(kt == 0), stop=(kt == DT - 1))
                    qze = sb.tile([P, N], BF16, tag=tag, bufs=4,
                                  name=f"{tag}e{mt}")
                    qzo = sb.tile([P, N], BF16, tag=tag + "o", bufs=4,
                                  name=f"{tag}o{mt}")
                    nc.gpsimd.memset(qze[DH:2 * DH, :], 0.0)
                    nc.gpsimd.memset(qzo[0:DH, :], 0.0)
                    nc.vector.tensor_scalar(out=qze[0:DH, :], in0=pq[0:DH, :],
                                            scalar1=bias_v[0:DH, mt:mt + 1],
                                            scalar2=None, op0=AL.add)
                    nc.vector.tensor_scalar(out=qzo[DH:2 * DH, :],
                                            in0=pq[DH:2 * DH, :],
                                            scalar1=bias_v[DH:2 * DH, mt:mt + 1],
                                            scalar2=None, op0=AL.add)
                    outs.append((qze, qzo))
                return outs

            qTs = proj_qk_padded(wq4, cv("bq"), "qT")
            kTs = proj_qk_padded(wk4, cv("bk"), "kT")

            # v in time-major layout with trailing ones columns per head
            bvv = cv("bvb")
            vext = []
            for jt in range(DT):
                pv = psp.tile([P, N], F32, tag="mm", bufs=2)
                for kt in range(DT):
                    nc.tensor.matmul(pv[:, :],
                                     zs[kt][:, jt * P:(jt + 1) * P],
                                     wv4[:, kt, :],
                                     start=(kt == 0), stop=(kt == DT - 1))
                vx = sb.tile([P, H * 2 * DH], BF16, tag="vext", bufs=4)
                vw = vx[:, 0:H * 2 * DH].rearrange("p (h c) -> p h c", c=2 * DH)
                nc.vector.scalar_tensor_tensor(
                    vw[:, :, 0:DH],
                    pv[:, :].rearrange("p (h d) -> p h d", h=H), 1.0,
                    bvv[:, :].rearrange("p (h d) -> p h d", h=H),
                    AL.mult, AL.add)
                nc.vector.tensor_copy(
                    vw[:, :, DH:2 * DH],
                    ones_v[:, None, 0:DH].broadcast_to([P, H, DH]))
                vext.append(vx)

            # prefetch conv weights now (transfers run during attention)
            c1ts = []
            for kt in range(DT):
                wt = sb.tile([P, 2 * CIN], BF16, tag="wst", bufs=5, name=f"c1t{kt}")
                nc.scalar.dma_start(wt[:, :], c1_d[kt * P:(kt + 1) * P, :])
                c1ts.append(wt)
            c2ts = []
            for q in range(2):
                wt = sb.tile([P, 4, DIM], BF16, tag="wst", bufs=5, name=f"c2t{q}")
                nc.scalar.dma_start(
                    wt[:, :, :],
                    bass.AP(c2_d, q * 4 * P * DIM,
                            [[DIM, P], [P * DIM, 4], [1, DIM]]))
                c2ts.append(wt)

            # qr bounce: qr = q @ relT -> DRAM (bf16), strided shift-gather
            # readback.  One write + one read dma_start per head (Pool queue).
            # Software-pipelined with the score computation (lookahead 2) so
            # no engine queue blocks on a buffer freed by work queued behind.
            rel4s = []
            oTs = [sb.tile([P, N], BF16, tag="oT", bufs=4, name=f"oTs{i}")
                   for i in range(DT)]

            def emit_qr_head(h):
                hb = (h % 2) * DH
                qt4 = sb.tile([P, DT, 640], BF16, tag="qtb", bufs=2,
                              name=f"qt4_{h}")
                qz = qTs[h // 2][h % 2]
                for it in range(DT):
                    lq = qz[:, it * P:(it + 1) * P]
                    cr0 = 3 * P - P * it
                    pq1 = psp.tile([P, 320], F32, tag="acc", bufs=4)
                    nc.tensor.matmul(pq1[:, :], lq,
                                     relTt[:, cr0:cr0 + 320],
                                     start=True, stop=True)
                    pq2 = psp.tile([P, 320], F32, tag="acc", bufs=4)
                    nc.tensor.matmul(pq2[:, :], lq,
                                     relTt[:, cr0 + 320:cr0 + 640],
                                     start=True, stop=True)
                    nc.vector.tensor_copy(qt4[:, it, 0:320], pq1[:, :])
                    nc.vector.tensor_copy(qt4[:, it, 320:640], pq2[:, :])
                # dst[p, it, c] = qr[it*128+p, (3-it)*128 + c]
                nc.gpsimd.dma_start(
                    bass.AP(qr_ds[h], 3 * P,
                            [[QRW, P], [P * (QRW - 1), DT], [1, 640]]),
                    qt4[:, :, :])
                # XBAR transpose-gather straight from the DRAM scratch:
                # relx[j', jt, i] = rel_tm[i, jt*128+j'] = qr[i, 512 - i + j]
                relx = sb.tile([P, DT, N], BF16, tag="relx", bufs=4,
                               name=f"relx_{h}")
                nc.sync.dma_start(
                    relx[:, :, :],
                    bass.AP(qr_ds[h], 4 * P, [[QRW - 1, N], [1, N]]),
                    transpose=True)
                rel4s.append(relx)

            exps_all = {}

            def emit_scores_chains(h):
                hb = (h % 2) * DH
                relx = rel4s[h]
                exps = []
                for jt in range(DT):
                    pss = psp.tile([P, N], F32, tag="acc", bufs=4)
                    nc.tensor.matmul(pss[:, :],
                                     kTs[h // 2][h % 2][:, jt * P:(jt + 1) * P],
                                     qTs[h // 2][h % 2][:, :],
                                     start=True, stop=False)
                    nc.tensor.matmul(pss[:, :], idbf[:, :], relx[:, jt, :],
                                     start=False, stop=True)
                    e = sb.tile([P, N], BF16, tag="exp", bufs=8)
                    nc.scalar.activation(e[:, :], pss[:, :], AF.Exp)
                    exps.append(e)
                exps_all[h] = exps

            def emit_po(h):
                hb = (h % 2) * DH
                exps = exps_all.pop(h)
                po = psp.tile([P, N], F32, tag="mm", bufs=2)
                for jt in range(DT):
                    nc.tensor.matmul(po[:, :],
                                     vext[jt][:, h * 2 * DH:(h + 1) * 2 * DH],
                                     exps[jt][:, :],
                                     start=(jt == 0), stop=(jt == DT - 1))
                lnd = sb.tile([DH, N], F32, tag="dwt", bufs=2)
                nc.scalar.activation(lnd[:, :], po[DH:2 * DH, :], AF.Ln)
                rcp = sb.tile([DH, N], F32, tag="dwt", bufs=2)
                nc.scalar.activation(rcp[:, :], lnd[:, :], AF.Exp, scale=-1.0)
                nc.vector.tensor_mul(oTs[h // 2][hb:hb + DH, :], po[0:DH, :],
                                     rcp[:, :])

            LA = 3
            for step in range(H + LA + 1):
                if step < H:
                    emit_qr_head(step)
                if LA <= step < H + LA:
                    emit_scores_chains(step - LA)
                if step >= LA + 1:
                    emit_po(step - LA - 1)

            # out-projection + residual
            pas = [psp.tile([P, N], F32, tag="acc", bufs=4, name=f"pas{i}")
                   for i in range(DT)]
            for kt in range(DT):
                for mt in range(DT):
                    nc.tensor.matmul(pas[mt][:, :],
                                     wo4[:, kt, mt * P:(mt + 1) * P],
                                     oTs[kt][:, :],
                                     start=(kt == 0), stop=(kt == DT - 1))
            bov = cv("bo")
            x2 = []
            x2bf = []
            for mt in range(DT):
                t = sb.tile([P, N], F32R, tag="x", bufs=6)
                nc.vector.scalar_tensor_tensor(t[:, :], pas[mt][:, :],
                                               bov[:, mt:mt + 1], x1[mt][:, :],
                                               AL.add, AL.add)
                x2.append(t)
                xb = sb.tile([P, N], BF16, tag="x2bf", bufs=4, name=f"x2bf{mt}")
                if mt % 2 == 0:
                    nc.scalar.copy(xb[:, :], t[:, :])
                else:
                    nc.vector.tensor_copy(xb[:, :], t[:, :])
                x2bf.append(xb)

            # ================= conv module =================
            # prefetch depthwise diagonal blocks + ff2 weights
            dgs = []
            for ct in range(CT):
                dg = sb.tile([P, KW * P], BF16, tag="dg", bufs=2, name=f"dg{ct}")
                nc.scalar.dma_start(dg[:, :], dwd_d[ct, :, :])
                dgs.append(dg)
            w3ts = []
            for kt in range(DT):
                wt = sb.tile([P, FF], BF16, tag="wst", bufs=5, name=f"w3t{kt}")
                nc.sync.dma_start(wt[:, :], w3_d[kt * P:(kt + 1) * P, :])
                w3ts.append(wt)
            w4ts = []
            for q in range(4):
                wt = sb.tile([P, 4, DIM], BF16, tag="w2t", bufs=4, name=f"w4t{q}")
                nc.scalar.dma_start(
                    wt[:, :, :],
                    bass.AP(w4_d, q * 4 * P * DIM,
                            [[DIM, P], [P * DIM, 4], [1, DIM]]))
                w4ts.append(wt)

            # conv1 + GLU (c1 host layout: columns [a(0:1024) | g(1024:2048)])
            c1av, c1gv = cv("c1a"), cv("c1g")
            glus = []
            for ct in range(CT):
                pa = psp.tile([P, N], F32, tag="acc", bufs=4)
                pg = psp.tile([P, N], F32, tag="acc", bufs=4)
                for kt in range(DT):
                    nc.tensor.matmul(pa[:, :],
                                     c1ts[kt][:, ct * P:(ct + 1) * P],
                                     x2bf[kt][:, :],
                                     start=(kt == 0), stop=(kt == DT - 1))
                for kt in range(DT):
                    nc.tensor.matmul(pg[:, :],
                                     c1ts[kt][:, CIN + ct * P:CIN + (ct + 1) * P],
                                     x2bf[kt][:, :],
                                     start=(kt == 0), stop=(kt == DT - 1))
                sig = sb.tile([P, N], F32, tag="tmp", bufs=3)
                nc.scalar.activation(sig[:, :], pg[:, :], AF.Sigmoid,
                                     bias=c1gv[:, ct:ct + 1], scale=1.0)
                glu = sb.tile([P, PAD + N], BF16, tag="glu", bufs=3)
                nc.vector.memset(glu[:, 0:PAD], 0.0)
                nc.vector.scalar_tensor_tensor(glu[:, PAD:PAD + N], pa[:, :],
                                               c1av[:, ct:ct + 1], sig[:, :],
                                               AL.add, AL.mult)
                glus.append(glu)

            # depthwise conv as 31 diagonal matmuls per channel block
            bnsv, bntv = cv("bns"), cv("bnt")
            hcs = []
            for ct in range(CT):
                pd = psp.tile([P, N], F32, tag="mm", bufs=2)
                for k in range(KW):
                    nc.tensor.matmul(pd[:, :], dgs[ct][:, k * P:(k + 1) * P],
                                     glus[ct][:, k:k + N],
                                     start=(k == 0), stop=(k == KW - 1))
                sig = sb.tile([P, N], F32, tag="dwt", bufs=2)
                nc.scalar.activation(sig[:, :], pd[:, :], AF.Sigmoid,
                                     bias=bntv[:, ct:ct + 1],
                                     scale=bnsv[:, ct:ct + 1])
                u = sb.tile([P, N], F32, tag="dwt", bufs=2)
                nc.vector.tensor_scalar(out=u[:, :], in0=pd[:, :],
                                        scalar1=bnsv[:, ct:ct + 1],
                                        scalar2=bntv[:, ct:ct + 1],
                                        op0=AL.mult, op1=AL.add)
                hc = sb.tile([P, N], BF16, tag="hc", bufs=6)
                nc.vector.tensor_mul(hc[:, :], u[:, :], sig[:, :])
                hcs.append(hc)

            # conv2 + residual (kt-outer)
            pcs = [psp.tile([P, N], F32, tag="acc", bufs=4, name=f"pcs{i}")
                   for i in range(DT)]
            for kt in range(CT):
                wv_ = c2ts[kt // 4][:, kt % 4, :]
                for mt in range(DT):
                    nc.tensor.matmul(pcs[mt][:, :],
                                     wv_[:, mt * P:(mt + 1) * P],
                                     hcs[kt][:, :],
                                     start=(kt == 0), stop=(kt == CT - 1))
            c2bv = cv("c2b")
            x3 = []
            for mt in range(DT):
                t = sb.tile([P, N], F32R, tag="x", bufs=6)
                nc.vector.scalar_tensor_tensor(t[:, :], pcs[mt][:, :],
                                               c2bv[:, mt:mt + 1], x2[mt][:, :],
                                               AL.add, AL.add)
                x3.append(t)

            # ================= ff2 =================
            x4 = ff_block(x3, w3ts, cv("b3"), w4ts, cv("b4"))

            # ================= post-LN (single output DMA) =================
            r_b, c_b = layer_norm_rc(x4)
            pngv, pnbv = cv("png"), cv("pnb")
            ot4 = sb.tile([P, DT, N], F32, tag="x4", bufs=1)
            for mt in range(DT):
                t = sb.tile([P, N], F32, tag="lnt", bufs=2)
                nc.vector.tensor_mul(t[:, :], x4[mt][:, :], r_b[:, :])
                t2 = sb.tile([P, N], F32, tag="mtile", bufs=1)
                nc.vector.tensor_add(t2[:, :], t[:, :], c_b[:, :])
                nc.vector.tensor_scalar(out=ot4[:, mt, :], in0=t2[:, :],
                                        scalar1=pngv[:, mt:mt + 1],
                                        scalar2=pnbv[:, mt:mt + 1],
                                        op0=AL.mult, op1=AL.add)
            for mt in range(DT):
                eng = nc.scalar if mt % 2 == 0 else nc.sync
                eng.dma_start(outT_d[mt * P:(mt + 1) * P, :].bitcast(F32),
                              ot4[:, mt, :])

    if split_waits:
        _split_matmul_waits(nc, mybir)
    return nc


def _split_matmul_waits(nc, mybir):
    """This walrus build rejects engine instructions carrying more than one
    sync wait; hoist the extras onto EventSemaphore instructions on the same
    engine queue right before the instruction."""
    fn = nc.m.functions[0]
    ctr = 0
    for blk in fn.blocks:
        out = []
        changed = False
        for ins in blk.instructions:
            si = ins.sync_info
            if (si is not None and si.on_wait and len(si.on_wait) > 1
                    and not isinstance(ins, (mybir.InstEventSemaphore,
                                             mybir.InstNoOp))):
                waits = list(si.on_wait)
                for w in waits[:-1]:
                    ev = mybir.InstNoOp(
                        name=f"EVW-{ctr}", ins=[], outs=[],
                        sync_info=mybir.SyncInfo(on_wait=[w], on_update=[]))
                    ev.engine = ins.engine
                    ctr += 1
                    out.append(ev)
                ins.sync_info = mybir.SyncInfo(
                    on_wait=[waits[-1]], on_update=list(si.on_update or []))
                changed = True
            out.append(ins)
        if changed:
            blk.instructions = out


def prep_inputs(inputs):
    """Host-side preprocessing: fold LN affines / scales / biases into weights."""
    import ml_dtypes

    f = np.float32
    bf = ml_dtypes.bfloat16
    ii = {k: np.asarray(v, dtype=f) for k, v in inputs.items()}

    def colmaj(b, nb):
        return np.ascontiguousarray(b.astype(f).reshape(nb, P).T)

    cstc = {}

    g1, be1 = ii["ff1_ln_g"], ii["ff1_ln_b"]
    w1 = np.ascontiguousarray((g1[:, None] * ii["ff1_w1"]).astype(bf))
    cstc["b1"] = colmaj(be1 @ ii["ff1_w1"] + ii["ff1_b1"], FT)
    w2 = np.ascontiguousarray((0.5 * ii["ff1_w2"]).astype(bf))
    cstc["b2"] = colmaj(0.5 * ii["ff1_b2"], DT)

    ag, ab = ii["attn_ln_g"], ii["attn_ln_b"]
    sc = DH ** -0.5
    wq = np.ascontiguousarray((ag[:, None] * ii["wq"] * sc).astype(bf))
    cstc["bq"] = colmaj((ab @ ii["wq"] + ii["bq"]) * sc, DT)
    wkv, bkv = ii["wkv"], ii["bkv"]
    wk = np.ascontiguousarray((ag[:, None] * wkv[:, :INNER]).astype(bf))
    cstc["bk"] = colmaj(ab @ wkv[:, :INNER] + bkv[:INNER], DT)
    wv = np.ascontiguousarray((ag[:, None] * wkv[:, INNER:]).astype(bf))
    cstc["bvb"] = np.ascontiguousarray(np.broadcast_to(
        ab @ wkv[:, INNER:] + bkv[INNER:], (P, INNER)))
    wo = np.ascontiguousarray(ii["wo"])
    cstc["bo"] = colmaj(ii["bo"], DT)
    # relT rows: head feature d lives at partition (h%2)*64 + d -> duplicate rows
    rT = ii["rel_emb"].T[:, ::-1]  # [64, 1025] column-reversed
    relT = np.ascontiguousarray(np.concatenate([rT, rT], axis=0).astype(bf))

    # c1 columns: [a(0:1024) | g(1024:2048)] straight
    c1 = np.ascontiguousarray(ii["conv1_w"].astype(bf))
    c1b = ii["conv1_b"]
    cstc["c1a"] = colmaj(c1b[:CIN], CT)
    cstc["c1g"] = colmaj(c1b[CIN:], CT)
    dwd = np.zeros((CT, P, KW, P), dtype=bf)
    wr = ii["dw_w"].reshape(CT, P, KW).astype(bf)
    pp = np.arange(P)
    for ct in range(CT):
        for k in range(KW):
            dwd[ct, pp, k, pp] = wr[ct, :, k]
    dwd = np.ascontiguousarray(dwd.reshape(CT, P, KW * P))
    inv = 1.0 / np.sqrt(ii["bn_var"] + EPS)
    s = inv * ii["bn_g"]
    t = ii["bn_b"] - ii["bn_mean"] * s
    cstc["bns"] = colmaj(s, CT)
    cstc["bnt"] = colmaj(t + s * ii["dw_b"], CT)
    c2 = np.ascontiguousarray(ii["conv2_w"].astype(bf))
    cstc["c2b"] = colmaj(ii["conv2_b"], DT)

    g3, be3 = ii["ff2_ln_g"], ii["ff2_ln_b"]
    w3 = np.ascontiguousarray((g3[:, None] * ii["ff2_w1"]).astype(bf))
    cstc["b3"] = colmaj(be3 @ ii["ff2_w1"] + ii["ff2_b1"], FT)
    w4 = np.ascontiguousarray((0.5 * ii["ff2_w2"]).astype(bf))
    cstc["b4"] = colmaj(0.5 * ii["ff2_b2"], DT)

    cstc["png"] = colmaj(ii["pn_g"], DT)
    cstc["pnb"] = colmaj(ii["pn_b"], DT)
    cstr = np.concatenate([np.ones((P, P), dtype=f), np.eye(P, dtype=f),
                           np.full((P, P), 1.0 / DIM, dtype=f)], axis=1)

    cstblob = np.zeros((P, CSTW), dtype=f)
    for name, (off, w) in CST_OFF.items():
        cstblob[:, off:off + w] = cstc[name]

    shared = dict(cst=np.ascontiguousarray(cstblob),
                  cstr=np.ascontiguousarray(cstr), relT=relT,
                  idbf=np.ascontiguousarray(np.eye(P, dtype=bf)),
                  w1=w1, w2=w2, wq=wq, wk=wk, wv=wv,
                  wo=np.ascontiguousarray(wo.astype(bf)),
                  c1=c1, dwdiag=dwd, c2=c2, w3=w3, w4=w4)
    x = ii["x"]
    in_maps = []
    for b in range(NCORES):
        m = dict(shared)
        m["xT"] = np.ascontiguousarray(x[b].T)
        in_maps.append(m)
    return in_maps


_BUILT = None


def run(inputs, trace=False):
    global _BUILT
    from concourse import bass_utils

    in_maps = prep_inputs(inputs)
    if _BUILT is None:
        _BUILT = build()
    res = bass_utils.run_bass_kernel_spmd(
        _BUILT, in_maps, core_ids=list(range(NCORES)), trace=trace)
    out = np.stack([np.asarray(r["outT"]).T for r in res.results])
    return np.ascontiguousarray(out.astype(np.float32)), res


def kernel(**inputs):
    out, _ = run(inputs, trace=False)
    return out

